# revision 50
# baseline (speedup 1.0000x reference)
"""Co-Guiding GAT forward (2 layers, 4 masked-MHA branches) on 8 Trainium2 cores.

Sharding: core c = 2*b + p handles batch b; p=0 computes the h_a stream
(branches a2a + b2a), p=1 the h_b stream (b2b + a2b). Each core runs both
layers; the h exchange between layers is a pairwise AllGather of the
transposed fp16 activations, overlapped with the partner-independent half of
layer 2.

Attention is computed in transposed score layout S^T[k, q] per head so softmax
needs no transposes. The exp+mask work is spread across all three elementwise
engines: most units run exp on ACT (with the multiplicative {0,1} mask applied
on DVE or GPSIMD), and a fraction run a Schraudolph-style exp on DVE
(int32(A*x+B) bit trick) with the bitcast+mask fused into a GPSIMD multiply.
Row sums come from an appended ones-column in the V matmul; normalization is a
direct reciprocal on the PSUM sum rows plus an SBUF-replicating DMA broadcast
(no DRAM roundtrip). LayerNorm rstd uses a Newton-iterated rsqrt bit hack on
DVE so the ACT engine runs exp exclusively (single activation table).
"""

import sys

for _p in ("/opt/trn_rl_repo",):
    if _p not in sys.path:
        sys.path.insert(0, _p)

import math

import numpy as np

import concourse.bass as bass
import concourse.mybir as mybir
import concourse.tile as tile
from concourse import bacc
from concourse.bass_utils import run_bass_kernel_spmd
from concourse.masks import make_identity

F32 = mybir.dt.float32
F16 = mybir.dt.float16
I32 = mybir.dt.int32
AF = mybir.ActivationFunctionType
OP = mybir.AluOpType

N_CORES = 8
B, N, D = 4, 1024, 256
H, DK = 8, 32
L = 2
P = 128
NT = N // P  # 8 row tiles
DT = D // P  # 2 feature tiles
EPS = 1e-5
SCALE = 1.0 / math.sqrt(DK)
SHIFT = -12.0  # exp(s/sqrt(dk) + SHIFT): keeps p in fp16 range

# Schraudolph exp: exp(x) ~= bitcast_f32(int32(A*x + B)).
SCH_LOG2E_SCALE = float((1 << 23) / math.log(2.0))
SCH_C = 486411.0  # minimax-ish bias correction
SCH_A = SCH_LOG2E_SCALE * SCALE
SCH_B = float(127 * (1 << 23)) - SCH_C + SCH_LOG2E_SCALE * SHIFT

RSQRT_MAGIC = 0x5F3759DF

# exp/mask engine assignment, by unit position g = hp*16 + kc*2 + hh within a
# branch (64 units). 'S': Schraudolph (DVE int-exp + GPSIMD bitcast*mask).
# 'A': ACT exp; mask on DVE ('D') or GPSIMD ('G') per MASK_PAT.
SCH_EVERY = 6  # g % SCH_EVERY == SCH_PHASE -> Schraudolph unit
SCH_PHASE = 5
MASK_PAT = ("D", "G", "D", "G", "D")  # cycle over ACT units

_CACHED_NC = None
_LAST_IN_MAPS = None


def build_nc(finalize=True, dbg=False):
    nc = bacc.Bacc("TRN2", target_bir_lowering=False, debug=False,
                   num_devices=N_CORES)

    # ---- per-core DRAM I/O ----
    x_d = nc.dram_tensor("x", [N, D], F32, kind="ExternalInput")
    xT_d = nc.dram_tensor("xT", [D, N], F16, kind="ExternalInput")
    yT_d = nc.dram_tensor("yT", [D, N], F16, kind="ExternalInput")
    mks_d = nc.dram_tensor("maskTs", [N, N], F16, kind="ExternalInput")
    mkc_d = nc.dram_tensor("maskTc", [N, N], F16, kind="ExternalInput")
    # weights packed [l, role, mat(q,k,v,o), kc, 128, dout]
    wts_d = nc.dram_tensor("wts", [L, 2, 4, DT, P, D], F16, kind="ExternalInput")
    brow_d = nc.dram_tensor("brow", [1, L, 2, 4, D], F16, kind="ExternalInput")
    lng_d = nc.dram_tensor("lng", [L, 2, D], F32, kind="ExternalInput")
    lnb_d = nc.dram_tensor("lnb", [L, 2, D], F32, kind="ExternalInput")
    sel_d = nc.dram_tensor("sel", [P, 2], F32, kind="ExternalInput")
    out_d = nc.dram_tensor("out", [N, D], F32, kind="ExternalOutput")

    with tile.TileContext(nc) as tc:
        with (
            tc.tile_pool(name="const", bufs=1) as cpool,
            tc.tile_pool(name="wts", bufs=4) as wpool,
            tc.tile_pool(name="trans", bufs=1) as tpool,
            tc.tile_pool(name="nat", bufs=2) as npool,
            tc.tile_pool(name="qk", bufs=6) as qkpool,
            tc.tile_pool(name="vsb", bufs=2) as vpool,
            tc.tile_pool(name="pt", bufs=6) as ptpool,
            tc.tile_pool(name="i32", bufs=1) as ipool,
            tc.tile_pool(name="att", bufs=2) as apool,
            tc.tile_pool(name="rs", bufs=1) as rspool,
            tc.tile_pool(name="rb", bufs=1) as rbpool,
            tc.tile_pool(name="ln", bufs=2) as lnpool,
            tc.tile_pool(name="tall", bufs=2) as tallpool,
            tc.tile_pool(name="xac", bufs=1) as xaccpool,
            tc.tile_pool(name="pbig", bufs=2, space="PSUM") as pbig,
            tc.tile_pool(name="pav", bufs=1, space="PSUM") as pav,
            tc.tile_pool(name="pout", bufs=2, space="PSUM") as pout,
            tc.tile_pool(name="dram", bufs=2, space="DRAM") as dpool,
        ):
            # ---------- constants / resident loads ----------
            ident = cpool.tile([P, P], F32, tag="ident")
            make_identity(nc, ident)
            ones = cpool.tile([1, 512], F16, tag="ones")
            nc.vector.memset(ones, 1.0)
            shift = cpool.tile([P, 1], F32, tag="shift")
            nc.vector.memset(shift, SHIFT)
            magic8 = cpool.tile([P, NT, 1], I32, tag="magic8")
            nc.vector.memset(magic8, RSQRT_MAGIC)
            c15 = cpool.tile([P, NT, 1], F32, tag="c15")
            nc.vector.memset(c15, 1.5)

            # input DMAs ordered by first use: xT/yT/brow feed the first
            # projection; masks are only needed once the first scores land
            xT1 = tpool.tile([P, DT, N], F16, tag="xT")
            nc.sync.dma_start(out=xT1, in_=xT_d.ap().rearrange(
                "(dt p) q -> p dt q", p=P))
            yT1 = tpool.tile([P, DT, N], F16, tag="yT")
            nc.sync.dma_start(out=yT1, in_=yT_d.ap().rearrange(
                "(dt p) q -> p dt q", p=P))
            brow = cpool.tile([1, L, 2, 4, D], F16, tag="brow")
            nc.sync.dma_start(out=brow, in_=brow_d.ap())

            # DMA order = first-use order: layer-0 weights, then the role-0
            # mask (needed ~15us in), then the rest
            w_t = {}
            mask_sb = {}
            for l in range(L):
                for role in range(2):
                    wt = wpool.tile([P, 4, DT, D], F16, tag="w",
                                    name=f"w{l}{role}")
                    nc.sync.dma_start(out=wt, in_=wts_d.ap()[l, role].rearrange(
                        "m kc p d -> p m kc d"))
                    w_t[(l, role)] = wt
                m = cpool.tile([P, NT, N], F16, tag=f"mask{l}", name=f"mask{l}")
                md = mks_d if l == 0 else mkc_d
                nc.sync.dma_start(out=m, in_=md.ap().rearrange(
                    "(kc p) q -> p kc q", p=P))
                mask_sb[l] = m

            sel = cpool.tile([P, 2], F32, tag="sel")
            nc.sync.dma_start(out=sel, in_=sel_d.ap())
            gB = cpool.tile([P, L, 2, D], F32, tag="gB")
            bB = cpool.tile([P, L, 2, D], F32, tag="bB")
            for t, src in ((gB, lng_d), (bB, lnb_d)):
                bc = bass.AP(tensor=src.ap().tensor, offset=0,
                             ap=[[0, P]] + list(src.ap().ap))
                nc.gpsimd.dma_start(out=t, in_=bc)

            orig_x = cpool.tile([P, NT, D], F32, tag="orig_x")
            nc.sync.dma_start(out=orig_x, in_=x_d.ap().rearrange(
                "(nt p) d -> p nt d", p=P))

            # ---------------- helpers ----------------
            def proj_mat(l, role, m_i, src, w, dst, mc):
                """one 128-row slab of a projection in transposed layout.
                Evicts each 512-column half as soon as it lands so the first
                attention units start before the whole projection is done."""
                ps = pbig.tile([P, N], F32, tag="pb", name=f"ps{m_i}{mc}")
                for qc in range(2):
                    o = ps[:, qc * 512:(qc + 1) * 512]
                    for kc in range(DT):
                        nc.tensor.matmul(
                            o, w[:, m_i, kc, mc * P:(mc + 1) * P],
                            src[:, kc, qc * 512:(qc + 1) * 512],
                            start=(kc == 0), stop=False)
                    nc.tensor.matmul(
                        o, brow[0:1, l, role, m_i, mc * P:(mc + 1) * P],
                        ones[0:1, 0:512], start=False, stop=True)
                    nc.vector.tensor_copy(
                        out=dst[:, mc, qc * 512:(qc + 1) * 512], in_=o)

            def proj_q(l, role, xT, w):
                qT = qkpool.tile([P, DT, N], F16, tag="qk")
                for mc in range(DT):
                    proj_mat(l, role, 0, xT, w, qT, mc)
                return qT

            def proj_qk0(l, role, xT, kvT, w):
                """q+k interleaved mc-wise: the heads-0-3 slabs (mc=0) of both
                q and k land first, so the first S^T can issue ~4 matmuls in"""
                qT = qkpool.tile([P, DT, N], F16, tag="qk")
                kT = qkpool.tile([P, DT, N], F16, tag="qk")
                for mc in range(DT):
                    proj_mat(l, role, 0, xT, w, qT, mc)
                    proj_mat(l, role, 1, kvT, w, kT, mc)
                return qT, kT

            def proj_kv(l, role, kvT, w):
                """kT [dout, q] fp16, v_sb [n, h, dk+1] fp16 (ones col)."""
                kT = qkpool.tile([P, DT, N], F16, tag="qk")
                for mc in range(DT):
                    proj_mat(l, role, 1, kvT, w, kT, mc)
                v_sb = proj_v(l, role, kvT, w)
                return kT, v_sb

            def proj_v(l, role, kvT, w):
                v_sb = vpool.tile([P, NT, H, DK + 1], F16, tag="v")
                nc.gpsimd.memset(v_sb[:, :, :, DK:DK + 1], 1.0)
                for g4 in range(2):
                    ps = pbig.tile([P, N], F32, tag="pb")
                    for sub in range(4):
                        nt = g4 * 4 + sub
                        o = ps[:, sub * D:(sub + 1) * D]
                        for kc in range(DT):
                            nc.tensor.matmul(
                                o, kvT[:, kc, nt * P:(nt + 1) * P],
                                w[:, 2, kc, :], start=(kc == 0), stop=False)
                        nc.tensor.matmul(
                            o, ones[0:1, 0:P], brow[0:1, l, role, 2, :],
                            start=False, stop=True)
                    nc.vector.tensor_copy(
                        out=v_sb[:, g4 * 4:(g4 + 1) * 4, :, 0:DK],
                        in_=ps.rearrange("p (s h d) -> p s h d", s=4, h=H))
                return v_sb

            def attention(role, qT, kT, v_sb, mids=None, out_state=None,
                          out_key=None):
                """Masked softmax attention; returns normalized attnT fp16.

                One continuous 64-unit software pipeline (S^T two units ahead
                of AV) so the exp engines never stall at head-pair group
                boundaries. mids[hp]() is issued just after group hp's second
                S^T — partner projections and the previous branch's out-proj
                interleave there. The AV accumulator is released fast (raw
                eviction on ACT + reciprocal on DVE); the per-query
                normalization multiply runs later, off the critical path, once
                the reciprocal row broadcast lands."""
                maskT = mask_sb[role]
                araw = apool.tile([P, DT, N], F16, tag="attnT")
                if out_state is not None:
                    out_state[out_key] = araw
                opses = {}
                pts = {}

                def produce(g):
                    hp, pos = g // 16, g % 16
                    kc, hh = pos // 2, pos % 2
                    h = hp * 2 + hh
                    th, oh = h // 4, (h % 4) * DK
                    sps = pbig.tile([P, N], F32, tag="pb")
                    for qc in range(2):
                        nc.tensor.matmul(
                            sps[:, qc * 512:(qc + 1) * 512],
                            kT[oh:oh + DK, th, kc * P:(kc + 1) * P],
                            qT[oh:oh + DK, th, qc * 512:(qc + 1) * 512],
                            start=True, stop=True, tile_position=(oh, 0))
                    pt = ptpool.tile([P, N], F16, tag="pt")
                    if g % SCH_EVERY == SCH_PHASE:
                        it = ipool.tile([P, N], I32, tag="i32")
                        nc.vector.tensor_scalar(
                            out=it, in0=sps, scalar1=SCH_A, scalar2=SCH_B,
                            op0=OP.mult, op1=OP.add)
                        nc.gpsimd.tensor_tensor(
                            out=pt, in0=it.bitcast(F32),
                            in1=maskT[:, kc, :], op=OP.mult)
                    else:
                        nc.scalar.activation(out=pt, in_=sps, func=AF.Exp,
                                             scale=SCALE, bias=shift)
                        a_idx = g - (g // SCH_EVERY) - (1 if g % SCH_EVERY > SCH_PHASE else 0)
                        eng = nc.vector if MASK_PAT[a_idx % len(MASK_PAT)] == "D" \
                            else nc.gpsimd
                        eng.tensor_mul(pt, pt, maskT[:, kc, :])
                    return pt

                def av(g):
                    hp, pos = g // 16, g % 16
                    kc, hh = pos // 2, pos % 2
                    h = hp * 2 + hh
                    pt = pts.pop(g)
                    for qc in range(2):
                        nc.tensor.matmul(
                            opses[hp][hh * 64:hh * 64 + 33,
                                      qc * 512:(qc + 1) * 512],
                            v_sb[:, kc, h, :],
                            pt[:, qc * 512:(qc + 1) * 512],
                            start=(kc == 0), stop=(kc == NT - 1))

                def norm(hp):
                    ops = opses.pop(hp)
                    # fast PSUM release: reciprocal of the sum rows + raw
                    # (unnormalized) eviction on ACT, then the accumulator is
                    # free; the normalize multiply happens in-place later
                    rs = rspool.tile([P, N], F32, tag="rs")
                    nc.vector.reciprocal_approx_fast(out=rs, in_=ops)
                    for hh in range(2):
                        h = hp * 2 + hh
                        th, oh = h // 4, (h % 4) * DK
                        nc.scalar.activation(
                            out=araw[oh:oh + DK, th, :],
                            in_=ops[hh * 64:hh * 64 + 32, :], func=AF.Copy)
                    rsd = dpool.tile([2, N], F32, tag="rsd")
                    nc.sync.dma_start(out=rsd[0:1, :], in_=rs[32:33, :])
                    nc.sync.dma_start(out=rsd[1:2, :], in_=rs[96:97, :])
                    # rb rows land at each head's araw partition offset so the
                    # normalize STT sees aligned inputs (verifier requirement)
                    rb = rbpool.tile([P, N], F32, tag="rb")
                    for hh in range(2):
                        h = hp * 2 + hh
                        oh = (h % 4) * DK
                        nc.sync.dma_start(
                            out=rb[oh:oh + DK, :],
                            in_=rsd[hh:hh + 1, :].partition_broadcast(DK))
                    for hh in range(2):
                        h = hp * 2 + hh
                        th, oh = h // 4, (h % 4) * DK
                        nc.vector.scalar_tensor_tensor(
                            out=araw[oh:oh + DK, th, :],
                            in0=araw[oh:oh + DK, th, :], scalar=1.0,
                            in1=rb[oh:oh + DK, :],
                            op0=OP.mult, op1=OP.mult)

                for g in range(64):
                    hp, pos = g // 16, g % 16
                    if pos == 0:
                        opses[hp] = pav.tile([P, N], F32, tag="pa", name=f"ops{hp}")
                    pts[g] = produce(g)
                    if g >= 2:
                        av(g - 2)
                        if (g - 2) % 16 == 15:
                            norm((g - 2) // 16)
                    # mids fire after this position's AV/norm so interleaved
                    # work that reads normalized heads can't wedge the PE queue
                    if mids is not None and (hp, pos) in mids:
                        mids[(hp, pos)]()
                av(62)
                av(63)
                norm(3)
                return araw

            def outproj_stage1(l, role, araw, w, x_nat):
                """first half of the out-proj (heads 0-3 contraction + bias)
                plus the residual add — issued inside the same branch's
                attention once those heads are normalized, so only the second
                contraction half remains after the attention finishes."""
                t_all = tallpool.tile([P, NT, D], F32, tag="t_all")
                for nt in range(NT):
                    ops = pout.tile([P, D], F32, tag="po")
                    nc.tensor.matmul(
                        ops, araw[:, 0, nt * P:(nt + 1) * P],
                        w[:, 3, 0, :], start=True, stop=False)
                    nc.tensor.matmul(
                        ops, ones[0:1, 0:P], brow[0:1, l, role, 3, :],
                        start=False, stop=True)
                    nc.vector.tensor_add(t_all[:, nt, :], ops, x_nat[:, nt, :])
                return t_all

            def outproj_ln(l, role, araw, w, x_nat, xacc, xnew, pair_cb=None,
                           t_all=None):
                """out-proj + bias + residual + LayerNorm; role 0 fills xacc,
                role 1 combines into xnew (relu, plus orig_x residual at l=1).
                LN rstd via Newton-iterated rsqrt bit hack (keeps ACT on Exp).
                pair_cb(np2) is issued after each finished pair of row tiles
                so layer-boundary transposes can start before the tail ends.
                With t_all from outproj_stage1, only the second contraction
                half runs here (accumulated into t_all on DVE)."""
                staged = t_all is not None
                if not staged:
                    t_all = tallpool.tile([P, NT, D], F32, tag="t_all")
                mvall = lnpool.tile([P, NT, 2], F32, tag="mvall")
                for np2 in range(NT // 2):
                    nt0 = 2 * np2
                    for nt in (nt0, nt0 + 1):
                        ops = pout.tile([P, D], F32, tag="po")
                        if staged:
                            nc.tensor.matmul(
                                ops, araw[:, 1, nt * P:(nt + 1) * P],
                                w[:, 3, 1, :], start=True, stop=True)
                            nc.vector.tensor_tensor(
                                out=t_all[:, nt, :], in0=t_all[:, nt, :],
                                in1=ops, op=OP.add)
                        else:
                            for kc2 in range(DT):
                                nc.tensor.matmul(
                                    ops, araw[:, kc2, nt * P:(nt + 1) * P],
                                    w[:, 3, kc2, :], start=(kc2 == 0), stop=False)
                            nc.tensor.matmul(
                                ops, ones[0:1, 0:P], brow[0:1, l, role, 3, :],
                                start=False, stop=True)
                            nc.vector.tensor_add(t_all[:, nt, :], ops,
                                                 x_nat[:, nt, :])
                    st = lnpool.tile([P, 2, 6], F32, tag="st")
                    nc.vector.bn_stats(out=st[:, 0, :], in_=t_all[:, nt0, :])
                    nc.vector.bn_stats(out=st[:, 1, :], in_=t_all[:, nt0 + 1, :])
                    nc.vector.bn_aggr(out=mvall[:, nt0, :], in_=st[:, 0, :])
                    nc.vector.bn_aggr(out=mvall[:, nt0 + 1, :], in_=st[:, 1, :])
                    # rstd = rsqrt(var+eps): bit hack + 2 Newton steps,
                    # all on DVE (scalar-AP/int-scalar/STT ops are DVE-only
                    # on real HW; GPSIMD only takes plain tensor_tensor).
                    ve = lnpool.tile([P, 2, 1], F32, tag="ve")
                    nc.vector.tensor_scalar_add(ve, mvall[:, nt0:nt0 + 2, 1:2],
                                                EPS)
                    sh = lnpool.tile([P, 2, 1], I32, tag="sh")
                    nc.vector.tensor_scalar(
                        out=sh, in0=ve.bitcast(I32), scalar1=1,
                        scalar2=None, op0=OP.logical_shift_right)
                    r = lnpool.tile([P, 2, 1], F32, tag="r")
                    nc.vector.tensor_tensor(out=r.bitcast(I32),
                                            in0=magic8[:, 0:2, :],
                                            in1=sh, op=OP.subtract)
                    for _ in range(2):
                        r2 = lnpool.tile([P, 2, 1], F32, tag="r2")
                        nc.vector.tensor_mul(r2, r, r)
                        nc.vector.tensor_mul(r2, r2, ve)
                        a = lnpool.tile([P, 2, 1], F32, tag="a")
                        nc.vector.scalar_tensor_tensor(
                            out=a, in0=r2, scalar=-0.5, in1=c15[:, 0:2, :],
                            op0=OP.mult, op1=OP.add)
                        rn = lnpool.tile([P, 2, 1], F32, tag="r")
                        nc.vector.tensor_mul(rn, r, a)
                        r = rn
                    for j, nt in enumerate((nt0, nt0 + 1)):
                        # AP-scalar (TensorScalarPtr) ops are DVE-only on HW
                        tc_ = lnpool.tile([P, D], F32, tag="tc")
                        nc.vector.scalar_tensor_tensor(
                            out=tc_, in0=t_all[:, nt, :],
                            scalar=mvall[:, nt, 0:1],
                            in1=gB[:, l, role, :], op0=OP.subtract, op1=OP.mult)
                        if role == 0:
                            # bB role-0 slot holds beta0+beta1 (host-folded)
                            nc.vector.scalar_tensor_tensor(
                                out=xacc[:, nt, :], in0=tc_,
                                scalar=r[:, j, :], in1=bB[:, l, 0, :],
                                op0=OP.mult, op1=OP.add)
                        else:
                            pre = lnpool.tile([P, D], F32, tag="pre")
                            nc.vector.scalar_tensor_tensor(
                                out=pre, in0=tc_, scalar=r[:, j, :],
                                in1=xacc[:, nt, :], op0=OP.mult, op1=OP.add)
                            if l == 0:
                                nc.vector.tensor_scalar_max(xnew[:, nt, :],
                                                            pre, 0.0)
                            else:
                                nc.vector.scalar_tensor_tensor(
                                    out=xnew[:, nt, :], in0=pre, scalar=0.0,
                                    in1=orig_x[:, nt, :], op0=OP.max, op1=OP.add)
                                nc.sync.dma_start(
                                    out=out_d.ap().rearrange(
                                        "(nt2 p) d -> p nt2 d", p=P)[:, nt, :],
                                    in_=xnew[:, nt, :])
                    if pair_cb is not None:
                        pair_cb(np2)

            # ---------------- layers ----------------
            x_nat, xT, yT = orig_x, xT1, yT1
            exch = {}
            for l in range(L):
                w0 = w_t[(l, 0)]
                w1 = w_t[(l, 1)]
                qT0, kT0 = proj_qk0(l, 0, xT, xT, w0)
                v0 = proj_v(l, 0, xT, w0)

                # branch-1 projections are issued inside attention(0):
                # the q projection early (partner-independent), the k/v
                # projections once the partner exchange has surely landed
                state = {}

                def mid0_q(l=l, w1=w1, xT=xT):
                    state["qT1"] = proj_q(l, 1, xT, w1)
                    if l == 1:
                        # prefetch the AllGather result on the SWDGE queue so
                        # it can't block the HWDGE queue used by softmax
                        g0T = qkpool.tile([P, DT, N], F16, tag="qk")
                        g1T = qkpool.tile([P, DT, N], F16, tag="qk")
                        nc.gpsimd.dma_start(
                            out=g0T, in_=exch["xgT_out"][0:D, :].rearrange(
                                "(dt p) q -> p dt q", p=P))
                        nc.gpsimd.dma_start(
                            out=g1T, in_=exch["xgT_out"][D:2 * D, :].rearrange(
                                "(dt p) q -> p dt q", p=P))
                        state["g0T"], state["g1T"] = g0T, g1T

                def mid0_kv(l=l, w1=w1):
                    kvT = yT
                    if l == 1:
                        # select partner's half (sel is 0/1 per core parity)
                        yT2 = tpool.tile([P, DT, N], F16, tag="yT")
                        nc.vector.tensor_scalar_mul(yT2, state["g0T"],
                                                    sel[:, 0:1])
                        nc.vector.scalar_tensor_tensor(
                            out=yT2, in0=state["g1T"], scalar=sel[:, 1:2],
                            in1=yT2, op0=OP.mult, op1=OP.add)
                        kvT = yT2
                    state["kT1"], state["v1"] = proj_kv(l, 1, kvT, w1)

                a0 = attention(0, qT0, kT0, v0,
                               mids={(1, 1): mid0_q,
                                     (2 if l == 0 else 3, 1): mid0_kv})

                xacc = xaccpool.tile([P, NT, D], F32, tag="xacc")
                xnew = npool.tile([P, NT, D], F32, tag="xnew")

                def mid1(l=l, a0=a0, w0=w0, x_nat=x_nat, xacc=xacc, xnew=xnew):
                    outproj_ln(l, 0, a0, w0, x_nat, xacc, xnew)

                def mid2(l=l, w1=w1, x_nat=x_nat):
                    state["t_all1"] = outproj_stage1(
                        l, 1, state["araw1"], w1, x_nat)

                a1 = attention(1, state["qT1"], state["kT1"], state["v1"],
                               mids={(1, 1): mid1, (2, 2): mid2},
                               out_state=state, out_key="araw1")

                if l == 0:
                    # transpose x_new (fp16) half-by-half as the out-proj tail
                    # produces it, then exchange with the pair core
                    xT2 = tpool.tile([P, DT, N], F16, tag="xT")

                    def pair_cb(np2, xnew=xnew, xT2=xT2):
                        if np2 not in (1, 3):
                            return
                        g2 = np2 // 2
                        for dt_i in range(DT):
                            ps = pbig.tile([P, N], F32, tag="pb")
                            for s4 in range(4):
                                nt = g2 * 4 + s4
                                nc.tensor.transpose(
                                    ps[:, s4 * P:(s4 + 1) * P],
                                    xnew[:, nt, dt_i * P:(dt_i + 1) * P], ident)
                            nc.vector.tensor_copy(
                                out=xT2[:, dt_i, g2 * 512:(g2 + 1) * 512],
                                in_=ps[:, 0:512])
                        if np2 == 3:
                            xgT_in = dpool.tile([D, N], F16, tag="xgin")
                            xgT_out = dpool.tile([2 * D, N], F16, tag="xgout")
                            nc.sync.dma_start(
                                out=xgT_in.rearrange("(dt p) q -> p dt q", p=P),
                                in_=xT2)
                            nc.gpsimd.collective_compute(
                                "AllGather", OP.bypass,
                                replica_groups=[[2 * i, 2 * i + 1]
                                                for i in range(4)],
                                ins=[xgT_in.opt()], outs=[xgT_out.opt()])
                            exch["xgT_out"] = xgT_out
                else:
                    pair_cb = None

                outproj_ln(l, 1, a1, w1, x_nat, xacc, xnew, pair_cb,
                           t_all=state["t_all1"])

                if l == 0:
                    x_nat, xT = xnew, xT2

    if finalize:
        nc.finalize()
    return nc


def kernel(h_a, h_b, adj_a, adj_b, adj_ab, adj_ba,
           Wq, bq, Wk, bk, Wv, bv, Wo, bo, ln_g, ln_b):
    global _CACHED_NC, _LAST_IN_MAPS
    h_a = np.asarray(h_a, np.float32)
    h_b = np.asarray(h_b, np.float32)
    arrs = dict(Wq=np.asarray(Wq, np.float32), Wk=np.asarray(Wk, np.float32),
                Wv=np.asarray(Wv, np.float32), Wo=np.asarray(Wo, np.float32),
                bq=np.asarray(bq, np.float32), bk=np.asarray(bk, np.float32),
                bv=np.asarray(bv, np.float32), bo=np.asarray(bo, np.float32),
                ln_g=np.asarray(ln_g, np.float32), ln_b=np.asarray(ln_b, np.float32))
    adjs = dict(a=np.asarray(adj_a), b=np.asarray(adj_b),
                ab=np.asarray(adj_ab), ba=np.asarray(adj_ba))

    if _CACHED_NC is None:
        _CACHED_NC = build_nc()
    nc = _CACHED_NC

    in_maps = []
    for c in range(N_CORES):
        b, p = c // 2, c % 2
        if p == 0:
            x, y = h_a[b], h_b[b]
            mself, mcross = adjs["a"][b], adjs["ba"][b]
            roles = (0, 3)  # a2a, b2a
        else:
            x, y = h_b[b], h_a[b]
            mself, mcross = adjs["b"][b], adjs["ab"][b]
            roles = (1, 2)  # b2b, a2b
        wts = np.empty((L, 2, 4, DT, P, D), np.float16)
        brow = np.empty((1, L, 2, 4, D), np.float16)
        lng = np.empty((L, 2, D), np.float32)
        lnb = np.empty((L, 2, D), np.float32)
        for l in range(L):
            for r, j in enumerate(roles):
                for m, (Wn, bn) in enumerate(
                        (("Wq", "bq"), ("Wk", "bk"), ("Wv", "bv"), ("Wo", "bo"))):
                    wts[l, r, m] = arrs[Wn][l, j].reshape(DT, P, D)
                    brow[0, l, r, m] = arrs[bn][l, j]
                lng[l, r] = arrs["ln_g"][l, j]
            # role-0 slot carries beta0+beta1 (the kernel folds role-1's LN
            # bias into the role-0 accumulator); role-1 slot is unused
            lnb[l, 0] = arrs["ln_b"][l, roles[0]] + arrs["ln_b"][l, roles[1]]
            lnb[l, 1] = 0.0
        sel = np.zeros((P, 2), np.float32)
        sel[:, 1 - p] = 1.0  # p=0 wants partner (slot1); p=1 wants slot0
        in_maps.append({
            "x": np.ascontiguousarray(x),
            "xT": np.ascontiguousarray(x.T).astype(np.float16),
            "yT": np.ascontiguousarray(y.T).astype(np.float16),
            "maskTs": np.ascontiguousarray(mself.T).astype(np.float16),
            "maskTc": np.ascontiguousarray(mcross.T).astype(np.float16),
            "wts": wts, "brow": brow, "lng": lng, "lnb": lnb, "sel": sel,
        })

    _LAST_IN_MAPS = in_maps
    res = run_bass_kernel_spmd(nc, in_maps, list(range(N_CORES)))
    out_a = np.stack([res.results[2 * b]["out"] for b in range(B)])
    out_b = np.stack([res.results[2 * b + 1]["out"] for b in range(B)])
    return out_a, out_b
